# revision 36
# baseline (speedup 1.0000x reference)
"""Trainium2 Bass kernel for the AnaphoricityScorer problem.

Data-parallel over the batch (mention) dimension across 8 NeuronCores.
Per core: 64 mentions x 50 antecedents = 3200 pair rows, r = ant*64 + m.

pair = [a, b, a*b, pw] @ W1 restructured as:
 - b@W1b:  Tb = am @ (32*W1b) precomputed bf16->fp16, sharded 250 rows/core
           + DRAM AllGather; rows gathered per pair and injected into the
           PSUM accumulation via fp16 matmul-by-identity (transpose+accum
           on the TensorEngine).
 - (a*b)@W1c: fp8e4 DoubleRow matmuls (2 k-tiles per instruction, 0.5
           cyc/row): gathered b (bf16) transposed on PE, multiplied by a
           broadcast on DVE with fp8 output, weights host-packed
           [128, 8, 1024] fp8 at scale 32.
 - a@W1a + b1: Ta' = mentions @ (32*W1a) + 32*b1 computed on device (bf16)
           and injected through a constant 0/1 selection matrix fused with
           the pw k-tile: stationary [W1d; Ta'], moving [pwT; S].
Everything accumulates at scale 32; the Lrelu activation applies
scale=1/32 before the nonlinearity (exact since Lrelu is positively
homogeneous), emitting bf16 h for the W2 layer.
"""

import os
import sys
from contextlib import ExitStack

import numpy as np
import ml_dtypes

for _p in ("/opt/trn_rl_repo",):
    if _p not in sys.path and os.path.isdir(_p):
        sys.path.insert(0, _p)

from concourse import bass, mybir  # noqa: E402
import concourse.tile as tile  # noqa: E402
from concourse.masks import make_identity  # noqa: E402
from concourse.bass_utils import run_bass_kernel_spmd  # noqa: E402

NM, BATCH, A, E, PW, HID, NCORES = 2000, 512, 50, 1024, 64, 1024, 8
BS = BATCH // NCORES
R = A * BS
SHARD = NM // NCORES
ALPHA, EPSILON = 0.01, 1e-07
SC = 32.0
F32 = mybir.dt.float32
F16 = mybir.dt.float16
BF16 = mybir.dt.bfloat16
F8E4 = mybir.dt.float8e4
I32 = mybir.dt.int32
CHUNK = 512
KE, NT = E // 128, HID // 128

_CH = [512, 512, 512, 512, 384, 384, 256, 128]
_RCS = [0, 512, 1024, 1536, 2048, 2432, 2816, 3072]
NCHUNK = len(_CH)


def _redistribute_waits(nc, helper_sems, limit=1):
    """Enforce <=1 sync wait per instruction (walrus limit on this build).

    Compute-engine instructions execute in-order on their engine stream, so
    excess waits hoist into single-wait InstEventSemaphore instructions
    spliced just before them. DMACopy instructions execute from concurrent
    DGE queue programs, so an engine-stream EventSem does NOT gate them:
    their waits are bridged through a per-engine helper semaphore - the
    EventSems consume the original waits on the engine stream and increment
    the helper; the DMA's single wait slot watches the helper's cumulative
    count. Helpers are decremented back to zero at the end so repeated
    executions of the loaded NEFF stay correct.
    """
    counter = [0]
    counts = {e: 0 for e in helper_sems}
    last_blk = None

    def mk_ev(engine, wait=None, update=None):
        ev = mybir.InstEventSemaphore(
            name=f"hoistw-{counter[0]}", ins=[], outs=[]
        )
        counter[0] += 1
        ev.engine = engine
        ev.sync_info = mybir.SyncInfo(
            on_wait=[wait] if wait else [], on_update=[update] if update else []
        )
        return ev

    for f in nc.m.functions:
        for blk in f.blocks:
            il = blk.instructions
            if il:
                last_blk = blk
            new_il = []
            changed = False
            for inst in il:
                si = inst.sync_info
                waits = list(si.on_wait) if si is not None else []
                if isinstance(inst, mybir.InstDMACopy) and len(waits) > limit:
                    h = helper_sems[inst.engine]
                    for i, w in enumerate(waits):
                        upd = None
                        if i == len(waits) - 1:
                            upd = mybir.SyncUpdate(
                                sync_type="semaphore",
                                id=h.num,
                                ant_name=h.name,
                                update_mode="sem-inc",
                                update_value=1,
                            )
                        new_il.append(mk_ev(inst.engine, w, upd))
                    counts[inst.engine] += 1
                    si.on_wait = [
                        mybir.SyncWait(
                            sync_type="semaphore",
                            id=h.num,
                            ant_name=h.name,
                            wait_mode="sem-ge-imm",
                            wait_value=counts[inst.engine],
                        )
                    ]
                    changed = True
                elif len(waits) > limit:
                    for w in waits[:-limit]:
                        new_il.append(mk_ev(inst.engine, w))
                    si.on_wait = waits[-limit:]
                    changed = True
                new_il.append(inst)
            if changed:
                blk.instructions = new_il

    if last_blk is not None:
        il = list(last_blk.instructions)
        added = False
        for eng, h in helper_sems.items():
            for _ in range(counts[eng]):
                il.append(
                    mk_ev(
                        eng,
                        None,
                        mybir.SyncUpdate(
                            sync_type="semaphore",
                            id=h.num,
                            ant_name=h.name,
                            update_mode="sem-dec",
                            update_value=1,
                        ),
                    )
                )
                added = True
        if added:
            last_blk.instructions = il


NTILES = R // 128  # 25 gather tiles


def build_nc():
    nc = bass.Bass("TRN2", target_bir_lowering=False, debug=False)
    am_d = nc.declare_dram_parameter("am", [NM, E], BF16, isOutput=False)
    amT_d = nc.declare_dram_parameter("amT", [128, KE * SHARD], BF16, isOutput=False)
    mT_d = nc.declare_dram_parameter("mT", [128, KE * BS], BF16, isOutput=False)
    pwS_d = nc.declare_dram_parameter("pwS", [128, R], BF16, isOutput=False)
    # cols 0..NTILES-1: gather indices; NTILES: arange(128);
    # NTILES+1 / NTILES+2: this core's Tb-shard scatter row offsets
    idx_d = nc.declare_dram_parameter("idx", [128, NTILES + 3], I32, isOutput=False)
    rough_d = nc.declare_dram_parameter("rough", [1, R], F32, isOutput=False)
    w1a_d = nc.declare_dram_parameter("w1a", [E, HID], BF16, isOutput=False)
    w1b_d = nc.declare_dram_parameter("w1b", [E, HID], BF16, isOutput=False)
    w1c8_d = nc.declare_dram_parameter("w1c8", [128, KE * HID], F8E4, isOutput=False)
    w1d_d = nc.declare_dram_parameter("w1d", [PW, HID], BF16, isOutput=False)
    w2r_d = nc.declare_dram_parameter("w2r", [128, NT], BF16, isOutput=False)
    b1r_d = nc.declare_dram_parameter("b1r", [1, HID], BF16, isOutput=False)
    b2s_d = nc.declare_dram_parameter("b2s", [1, 1], F32, isOutput=False)
    out_d = nc.declare_dram_parameter("out", [1, R], F32, isOutput=True)

    helper_sems = {
        mybir.EngineType.SP: nc.alloc_semaphore("hoist_dma_sp"),
        mybir.EngineType.Pool: nc.alloc_semaphore("hoist_dma_pool"),
        mybir.EngineType.Activation: nc.alloc_semaphore("hoist_dma_act"),
        mybir.EngineType.DVE: nc.alloc_semaphore("hoist_dma_dve"),
    }

    with tile.TileContext(nc) as tc:
        with ExitStack() as ctx:
            const = ctx.enter_context(tc.tile_pool(name="const", bufs=1))
            dram = ctx.enter_context(tc.tile_pool(name="dram", bufs=1, space="DRAM"))

            w1c8_sb = const.tile([128, KE, HID], F8E4, tag="w1c8")
            wfused = const.tile([128, HID], BF16, tag="wfused")
            pwS_sb = const.tile([128, R], BF16, tag="pwS")
            mt_all = const.tile([128, KE, BS], BF16, tag="mt_all")
            it_all = const.tile([128, NTILES + 3], I32, tag="it_all")
            w2_sb = const.tile([128, NT], BF16, tag="w2")
            b2_sb = const.tile([1, 1], F32, tag="b2")
            identf = const.tile([128, 128], F32, tag="identf")
            identb = const.tile([128, 128], BF16, tag="identb")
            ident16 = const.tile([128, 128], F16, tag="ident16")
            ones_f = const.tile([1, BS], F32, tag="ones_f")
            ones1 = const.tile([1, BS], BF16, tag="ones1")

            make_identity(nc, identf[:])
            nc.vector.tensor_copy(identb[:], identf[:])
            nc.vector.tensor_copy(ident16[:], identf[:])
            nc.gpsimd.memset(ones_f[:], 1.0)
            nc.vector.tensor_copy(ones1[:], ones_f[:])

            # const loads on the Activation-engine DMA queue; idx first so
            # gathers can start immediately
            nc.scalar.dma_start(it_all[:], idx_d[:])
            nc.scalar.dma_start(mt_all[:].rearrange("p e n -> p (e n)"), mT_d[:])
            nc.scalar.dma_start(w2_sb[:], w2r_d[:])
            nc.scalar.dma_start(b2_sb[:], b2s_d[:])
            nc.scalar.dma_start(wfused[0:PW, :], w1d_d[:])

            def mts(e):
                return mt_all[:, e, :]

            tb_full = dram.tile([NM, HID], F16, tag="tbf", addr_space="Shared")
            tb_bounce = dram.tile([SHARD, HID], F16, tag="tbb")

            gb_pool = ctx.enter_context(tc.tile_pool(name="gb", bufs=NTILES))
            gt_pool = ctx.enter_context(tc.tile_pool(name="gt", bufs=16))
            abT_pool = ctx.enter_context(tc.tile_pool(name="abT", bufs=2))
            h_pool = ctx.enter_context(tc.tile_pool(name="h", bufs=4))
            hpre_pool = ctx.enter_context(tc.tile_pool(name="hpre", bufs=3 * NT))
            o_pool = ctx.enter_context(tc.tile_pool(name="o", bufs=2))
            rough_pool = ctx.enter_context(tc.tile_pool(name="rough", bufs=2))

            def emit_gb(c, rc, NC):
                gbs = []
                for t in range(NC // 128):
                    tg = rc // 128 + t
                    gb = gb_pool.tile([128, E], BF16, tag="gb", name=f"gb{c}_{t}")
                    nc.gpsimd.indirect_dma_start(
                        out=gb[:], out_offset=None, in_=am_d[:],
                        in_offset=bass.IndirectOffsetOnAxis(ap=it_all[:, tg : tg + 1], axis=0))
                    gbs.append(gb)
                abT = abT_pool.tile([128, KE, CHUNK], F8E4, tag="abT", name=f"abT{c}")
                return abT, gbs

            def emit_gt(c, rc, NC):
                gts = []
                for t in range(NC // 128):
                    tg = rc // 128 + t
                    gt = gt_pool.tile([128, E], F16, tag="gt", name=f"gt{c}_{t}")
                    nc.gpsimd.indirect_dma_start(
                        out=gt[:], out_offset=None, in_=tb_full[:],
                        in_offset=bass.IndirectOffsetOnAxis(ap=it_all[:, tg : tg + 1], axis=0))
                    gts.append(gt)
                return gts

            def transpose_unit(c, abT, gbs, t, e):
                tp = tp_pool.tile([128, 128], BF16, tag="tp", space="PSUM", name=f"tp{c}_{t}_{e}")
                nc.tensor.transpose(tp[:], gbs[t][:, 128 * e : 128 * (e + 1)], identb[:])
                sl = slice(128 * t, 128 * (t + 1))
                nc.vector.tensor_tensor(
                    out=abT[:, e, sl].rearrange("p (u m) -> p u m", m=BS),
                    in0=tp[:].rearrange("p (u m) -> p u m", m=BS),
                    in1=mt_all[:, e : e + 1, :].to_broadcast([128, 2, BS]),
                    op=mybir.AluOpType.mult)

            wa_pool = ctx.enter_context(tc.tile_pool(name="wa", bufs=8))
            b1_pool = ctx.enter_context(tc.tile_pool(name="b1p", bufs=1))
            b1_sb = b1_pool.tile([1, HID], BF16, tag="b1")
            nc.scalar.dma_start(b1_sb[:], b1r_d[:])
            was = []
            for k in range(KE):
                wa_t = wa_pool.tile([128, HID], BF16, tag="wa", name=f"wa{k}")
                nc.scalar.dma_start(wa_t[:], w1a_d[128 * k : 128 * (k + 1), :])
                was.append(wa_t)
            nc.scalar.dma_start(
                w1c8_sb[:].rearrange("p e n -> p (e n)"), w1c8_d[:]
            )
            nc.scalar.dma_start(pwS_sb[:], pwS_d[:])

            # ---- Phase T: Tb shard = amT_shard.T @ (32*W1b) -> fp16,
            # parity-packed so the whole 250-row shard scatters into the
            # SHARED table with ONE indirect DMA (row-pair granularity).
            HSH = SHARD // 2  # 125 row-pairs
            with tc.tile_pool(name="wb", bufs=8) as wb_pool, tc.tile_pool(
                name="ptb", bufs=4, space="PSUM"
            ) as ptb_pool, tc.tile_pool(name="tbsb", bufs=1) as tbsb_pool:
                amT_sb = tbsb_pool.tile([128, 2 * KE, HSH], BF16, tag="amT")
                nc.sync.dma_start(amT_sb[:].rearrange("p e m -> p (e m)"), amT_d[:])
                wbs = []
                for k in range(KE):
                    wb_t = wb_pool.tile([128, HID], BF16, tag="wb", name=f"wb{k}")
                    nc.sync.dma_start(wb_t[:], w1b_d[128 * k : 128 * (k + 1), :])
                    wbs.append(wb_t)
                tb_sb = tbsb_pool.tile([128, 2 * HID], F16, tag="tbsb")
                for j in range(2):
                    jsl = slice(512 * j, 512 * (j + 1))
                    for par in range(2):
                        ps_tb = ptb_pool.tile([128, 512], F32, tag="ptb", name=f"ps_tb{j}_{par}")[0:HSH, :]
                        for k in range(KE):
                            nc.tensor.matmul(
                                ps_tb[:], amT_sb[:, 2 * k + par, :], wbs[k][:, jsl],
                                start=(k == 0), stop=(k == KE - 1),
                            )
                        nc.vector.tensor_copy(
                            tb_sb[0:HSH, 1024 * par + 512 * j : 1024 * par + 512 * (j + 1)],
                            ps_tb[:],
                        )

                # one contiguous bounce write (row pairs == contiguous rows)
                nc.sync.dma_start(
                    tb_bounce[:].rearrange("(a b) c -> a (b c)", b=2),
                    tb_sb[0:HSH, :],
                )
                # all b-gathers BEFORE the collective (they flow while the
                # collective waits for its input), then the collective, then
                # all Tb gathers (their waits then block nothing upstream).
                gb_all = [emit_gb(c, _RCS[c], _CH[c]) for c in range(2)]
                nc.gpsimd.collective_compute(
                    "AllGather",
                    mybir.AluOpType.bypass,
                    replica_groups=[list(range(NCORES))],
                    ins=[tb_bounce[:]],
                    outs=[tb_full[:]],
                )
                for c in range(2, NCHUNK):
                    gb_all.append(emit_gb(c, _RCS[c], _CH[c]))
                gt_all = [emit_gt(c, _RCS[c], _CH[c]) for c in range(NCHUNK)]

            # ---- Phase A: Ta' = 32*(mentions @ W1a + b1) -> wfused[64:128, :]
            with tc.tile_pool(name="pta", bufs=2, space="PSUM") as pta_pool:
                for j in range(2):
                    jsl = slice(512 * j, 512 * (j + 1))
                    ps_ta = pta_pool.tile([128, 512], F32, tag="pta", name=f"ps_ta{j}")[0:BS, :]
                    nc.tensor.matmul(ps_ta[:], ones1[0:1, :], b1_sb[0:1, jsl], start=True, stop=False)
                    for k in range(KE):
                        nc.tensor.matmul(ps_ta[:], mts(k), was[k][:, jsl], start=False, stop=(k == KE - 1))
                    nc.vector.tensor_copy(wfused[PW : PW + BS, jsl], ps_ta[:])

            tp_pool = ctx.enter_context(tc.tile_pool(name="tp", bufs=3, space="PSUM"))
            psH = ctx.enter_context(tc.tile_pool(name="psH", bufs=4, space="PSUM"))
            psF = ctx.enter_context(tc.tile_pool(name="psF", bufs=1, space="PSUM"))

            # chunk 0: all transposes upfront
            for t in range(_CH[0] // 128):
                for e in range(KE):
                    transpose_unit(0, gb_all[0][0], gb_all[0][1], t, e)

            def emit_epilogue(c, rc, NC, ps_f):
                rough_t = rough_pool.tile([1, CHUNK], F32, tag="rough", name=f"ro{c}")
                nc.scalar.dma_start(rough_t[0:1, :NC], rough_d[0:1, rc : rc + NC])
                o_t = o_pool.tile([1, CHUNK], F32, tag="o", name=f"o{c}")
                nc.vector.tensor_tensor(out=o_t[0:1, :NC], in0=ps_f[0:1, :NC], in1=rough_t[0:1, :NC], op=mybir.AluOpType.add)
                nc.vector.tensor_scalar_add(o_t[0:1, :NC], o_t[0:1, :NC], b2_sb[0:1, 0:1])
                nc.sync.dma_start(out_d[0:1, rc : rc + NC], o_t[0:1, :NC])

            def emit_finish(c, rc, NC, n, ps_h, gts, ps_f):
                """Tb-injects + Lrelu + W2 for one (chunk, n-slice) group."""
                NCt = NC // 128
                nsl = slice(128 * n, 128 * (n + 1))
                for t in range(NCt):
                    nc.tensor.matmul(
                        ps_h[:, 128 * t : 128 * (t + 1)],
                        gts[t][:, nsl], ident16[:],
                        start=False, stop=(t == NCt - 1),
                    )
                h_t = h_pool.tile([128, CHUNK], BF16, tag="h", name=f"h{c}_{n}")
                nc.scalar.activation(
                    h_t[:, :NC], ps_h[:, :NC],
                    mybir.ActivationFunctionType.Lrelu,
                    alpha=ALPHA, scale=1.0 / SC,
                )
                nc.tensor.matmul(ps_f[0:1, :NC], w2_sb[:, n : n + 1], h_t[:, :NC], start=(n == 0), stop=(n == NT - 1))

            def emit_main(c, rc, NC, n, abT, defer):
                """pw/Ta-fused + fp8 DR matmuls for one (chunk, n) group.
                defer=True closes the group and parks it in bf16 h_pre."""
                nsl = slice(128 * n, 128 * (n + 1))
                halves = [(0, NC)]
                ps_h = psH.tile([128, CHUNK], F32, tag="ps_h", name=f"ps_h{c}_{n}")
                # open the bank full-width, then accumulate
                nc.tensor.matmul(ps_h[:, :NC], wfused[:, nsl], pwS_sb[:, rc : rc + NC], start=True, stop=False)
                for k2 in range(KE // 2):
                    esl = slice(2 * k2, 2 * k2 + 2)
                    last = k2 == KE // 2 - 1
                    for hi, (h0, hw) in enumerate(halves):
                        nc.tensor.matmul(
                            ps_h[:, h0 : h0 + hw],
                            w1c8_sb[:, esl, nsl],
                            abT[:, esl, h0 : h0 + hw],
                            start=False, stop=(defer and last and hi == len(halves) - 1),
                            perf_mode=mybir.MatmulPerfMode.DoubleRow,
                        )
                if defer:
                    h_pre = hpre_pool.tile([128, CHUNK], BF16, tag="hpre", name=f"hpre{c}_{n}")
                    nc.vector.tensor_copy(h_pre[:, :NC], ps_h[:, :NC])
                    return h_pre
                return ps_h

            # ---- chunks 0-1: DR+fused only, parked in h_pre (no dependency
            # on the collective); interleave next chunk's transposes
            N_DEFER = 3
            hpres = []
            for c in range(N_DEFER):
                rc, NC = _RCS[c], _CH[c]
                units = [(t, e) for t in range(_CH[c + 1] // 128) for e in range(KE)]
                per_group = (len(units) + NT - 1) // NT
                hp = []
                for n in range(NT):
                    hp.append(emit_main(c, rc, NC, n, gb_all[c][0], defer=True))
                    for _ in range(per_group):
                        if units:
                            t, e = units.pop(0)
                            transpose_unit(c + 1, gb_all[c + 1][0], gb_all[c + 1][1], t, e)
                hpres.append(hp)

            # ---- deferred finish of chunks 0-1: re-inject h_pre, add Tb,
            # activate, W2, epilogue
            for c in range(N_DEFER):
                rc, NC = _RCS[c], _CH[c]
                ps_f = psF.tile([1, CHUNK], F32, tag="ps_f", name=f"ps_f{c}")
                for n in range(NT):
                    ps_h = psH.tile([128, CHUNK], F32, tag="ps_h", name=f"ps_hd{c}_{n}")
                    nc.tensor.matmul(ps_h[:, :NC], identb[:], hpres[c][n][:, :NC], start=True, stop=False)
                    emit_finish(c, rc, NC, n, ps_h, gt_all[c], ps_f)
                emit_epilogue(c, rc, NC, ps_f)

            # ---- chunks 2+: normal fused flow
            for c in range(N_DEFER, NCHUNK):
                rc, NC = _RCS[c], _CH[c]
                gts = gt_all[c]
                if c + 1 < NCHUNK:
                    units = [(t, e) for t in range(_CH[c + 1] // 128) for e in range(KE)]
                else:
                    units = []
                per_group = (len(units) + NT - 1) // NT if units else 0
                ps_f = psF.tile([1, CHUNK], F32, tag="ps_f", name=f"ps_f{c}")
                for n in range(NT):
                    ps_h = emit_main(c, rc, NC, n, gb_all[c][0], defer=False)
                    emit_finish(c, rc, NC, n, ps_h, gts, ps_f)
                    for _ in range(per_group):
                        if units:
                            t, e = units.pop(0)
                            transpose_unit(c + 1, gb_all[c + 1][0], gb_all[c + 1][1], t, e)
                emit_epilogue(c, rc, NC, ps_f)

    _redistribute_waits(nc, helper_sems)
    return nc


_NC_CACHE = None


def _get_nc():
    global _NC_CACHE
    if _NC_CACHE is None:
        _NC_CACHE = build_nc()
    return _NC_CACHE


BF = ml_dtypes.bfloat16
F8 = ml_dtypes.float8_e4m3


def make_in_maps(
    all_mentions,
    mentions_batch,
    pw_batch,
    top_indices_batch,
    top_rough_scores_batch,
    W1,
    b1,
    W2,
    b2,
):
    am = np.asarray(all_mentions, np.float32)
    men = np.asarray(mentions_batch, np.float32)
    pw = np.asarray(pw_batch, np.float32)
    idx = np.asarray(top_indices_batch).astype(np.int32)
    rough = np.asarray(top_rough_scores_batch, np.float32)
    W1 = np.asarray(W1, np.float32)
    b1 = np.asarray(b1, np.float32)
    W2 = np.asarray(W2, np.float32)
    b2 = np.asarray(b2, np.float32)

    am_bf = am.astype(BF)
    w1a = np.ascontiguousarray((SC * W1[0:E]).astype(BF))
    w1b = np.ascontiguousarray((SC * W1[E : 2 * E]).astype(BF))
    w1c8 = np.ascontiguousarray(
        (SC * W1[2 * E : 3 * E]).reshape(KE, 128, HID).transpose(1, 0, 2).reshape(128, KE * HID)
    ).astype(F8)
    w1d = np.ascontiguousarray((SC * W1[3 * E : 3 * E + PW]).astype(BF))
    w2r = np.ascontiguousarray(W2[:, 0].reshape(NT, 128).T.astype(BF))
    b1r = np.ascontiguousarray((SC * b1).reshape(1, HID).astype(BF))
    b2s = np.ascontiguousarray(b2.reshape(1, 1))
    S = np.tile(np.eye(BS, dtype=np.float32), (1, A))

    in_maps = []
    for c in range(NCORES):
        sl = slice(c * BS, (c + 1) * BS)
        # [128, KE*BS]: mT[p, k*BS+m] = men[c*BS+m, 128k+p]
        mT = np.ascontiguousarray(
            men[sl].T.astype(BF).reshape(KE, 128, BS).transpose(1, 0, 2).reshape(128, KE * BS)
        )
        # parity-packed: amT[p, (2k+par)*125 + m'] = am[c*SHARD + 2m'+par, 128k+p]
        sh = am_bf[c * SHARD : (c + 1) * SHARD].T.reshape(KE, 128, SHARD // 2, 2)
        amT = np.ascontiguousarray(
            sh.transpose(1, 0, 3, 2).reshape(128, KE * SHARD)
        )
        pwT = pw[sl].transpose(2, 1, 0).reshape(PW, R)
        pwS = np.ascontiguousarray(np.concatenate([pwT, S], axis=0).astype(BF))
        # [128, NTILES+3] column-major per 128-row tile, then arange and
        # the Tb-shard scatter row offsets for this core
        cols = np.empty((128, NTILES + 3), np.int32)
        cols[:, :NTILES] = idx[sl].T.reshape(R).reshape(NTILES, 128).T
        ar = np.arange(128, dtype=np.int32)
        cols[:, NTILES] = ar
        # row-PAIR offsets into the [1000, 2048] view of the Tb table
        cols[:, NTILES + 1] = np.minimum(c * (SHARD // 2) + ar, NM // 2 - 1)
        cols[:, NTILES + 2] = 0
        idx_r = np.ascontiguousarray(cols)
        rough_r = np.ascontiguousarray(rough[sl].T.reshape(1, R))
        in_maps.append(
            dict(
                am=am_bf,
                amT=amT,
                mT=mT,
                pwS=pwS,
                idx=idx_r,
                rough=rough_r,
                w1a=w1a,
                w1b=w1b,
                w1c8=w1c8,
                w1d=w1d,
                w2r=w2r,
                b1r=b1r,
                b2s=b2s,
            )
        )
    return in_maps


def assemble_output(results):
    scores = np.empty((BATCH, A), np.float32)
    for c in range(NCORES):
        score_r = np.asarray(results[c]["out"]).reshape(A, BS)
        scores[c * BS : (c + 1) * BS, :] = score_r.T
    out = np.empty((BATCH, A + 1), np.float32)
    out[:, 0] = EPSILON
    out[:, 1:] = scores
    return out


def kernel(**inputs):
    nc = _get_nc()
    in_maps = make_in_maps(**inputs)
    res = run_bass_kernel_spmd(nc, in_maps, core_ids=list(range(NCORES)))
    return assemble_output(res.results)


if __name__ == "__main__":
    nc = build_nc()
    print("built ok")


# revision 37
# speedup vs baseline: 1.0674x; 1.0674x over previous
"""Trainium2 Bass kernel for the AnaphoricityScorer problem.

Data-parallel over the batch (mention) dimension across 8 NeuronCores.
Per core: 64 mentions x 50 antecedents = 3200 pair rows, r = ant*64 + m.

pair = [a, b, a*b, pw] @ W1 restructured as:
 - b@W1b:  Tb = am @ (32*W1b) precomputed bf16->fp16, sharded 250 rows/core
           + DRAM AllGather; rows gathered per pair and injected into the
           PSUM accumulation via fp16 matmul-by-identity (transpose+accum
           on the TensorEngine).
 - (a*b)@W1c: fp8e4 DoubleRow matmuls (2 k-tiles per instruction, 0.5
           cyc/row): gathered b (bf16) transposed on PE, multiplied by a
           broadcast on DVE with fp8 output, weights host-packed
           [128, 8, 1024] fp8 at scale 32.
 - a@W1a + b1: Ta' = mentions @ (32*W1a) + 32*b1 computed on device (bf16)
           and injected through a constant 0/1 selection matrix fused with
           the pw k-tile: stationary [W1d; Ta'], moving [pwT; S].
Everything accumulates at scale 32; the Lrelu activation applies
scale=1/32 before the nonlinearity (exact since Lrelu is positively
homogeneous), emitting bf16 h for the W2 layer.
"""

import os
import sys
from contextlib import ExitStack

import numpy as np
import ml_dtypes

for _p in ("/opt/trn_rl_repo",):
    if _p not in sys.path and os.path.isdir(_p):
        sys.path.insert(0, _p)

from concourse import bass, mybir  # noqa: E402
import concourse.tile as tile  # noqa: E402
from concourse.masks import make_identity  # noqa: E402
from concourse.bass_utils import run_bass_kernel_spmd  # noqa: E402

NM, BATCH, A, E, PW, HID, NCORES = 2000, 512, 50, 1024, 64, 1024, 8
BS = BATCH // NCORES
R = A * BS
SHARD = NM // NCORES
ALPHA, EPSILON = 0.01, 1e-07
SC = 32.0
F32 = mybir.dt.float32
F16 = mybir.dt.float16
BF16 = mybir.dt.bfloat16
F8E4 = mybir.dt.float8e4
I32 = mybir.dt.int32
CHUNK = 512
KE, NT = E // 128, HID // 128

_CH = [512, 512, 512, 512, 384, 384, 256, 128]
_RCS = [0, 512, 1024, 1536, 2048, 2432, 2816, 3072]
NCHUNK = len(_CH)


def _redistribute_waits(nc, helper_sems, limit=1):
    """Enforce <=1 sync wait per instruction (walrus limit on this build).

    Compute-engine instructions execute in-order on their engine stream, so
    excess waits hoist into single-wait InstEventSemaphore instructions
    spliced just before them. DMACopy instructions execute from concurrent
    DGE queue programs, so an engine-stream EventSem does NOT gate them:
    their waits are bridged through a per-engine helper semaphore - the
    EventSems consume the original waits on the engine stream and increment
    the helper; the DMA's single wait slot watches the helper's cumulative
    count. Helpers are decremented back to zero at the end so repeated
    executions of the loaded NEFF stay correct.
    """
    counter = [0]
    counts = {e: 0 for e in helper_sems}
    last_blk = None

    def mk_ev(engine, wait=None, update=None):
        ev = mybir.InstEventSemaphore(
            name=f"hoistw-{counter[0]}", ins=[], outs=[]
        )
        counter[0] += 1
        ev.engine = engine
        ev.sync_info = mybir.SyncInfo(
            on_wait=[wait] if wait else [], on_update=[update] if update else []
        )
        return ev

    for f in nc.m.functions:
        for blk in f.blocks:
            il = blk.instructions
            if il:
                last_blk = blk
            new_il = []
            changed = False
            for inst in il:
                si = inst.sync_info
                waits = list(si.on_wait) if si is not None else []
                if isinstance(inst, mybir.InstDMACopy) and len(waits) > limit:
                    h = helper_sems[inst.engine]
                    for i, w in enumerate(waits):
                        upd = None
                        if i == len(waits) - 1:
                            upd = mybir.SyncUpdate(
                                sync_type="semaphore",
                                id=h.num,
                                ant_name=h.name,
                                update_mode="sem-inc",
                                update_value=1,
                            )
                        new_il.append(mk_ev(inst.engine, w, upd))
                    counts[inst.engine] += 1
                    si.on_wait = [
                        mybir.SyncWait(
                            sync_type="semaphore",
                            id=h.num,
                            ant_name=h.name,
                            wait_mode="sem-ge-imm",
                            wait_value=counts[inst.engine],
                        )
                    ]
                    changed = True
                elif len(waits) > limit:
                    for w in waits[:-limit]:
                        new_il.append(mk_ev(inst.engine, w))
                    si.on_wait = waits[-limit:]
                    changed = True
                new_il.append(inst)
            if changed:
                blk.instructions = new_il

    if last_blk is not None:
        il = list(last_blk.instructions)
        added = False
        for eng, h in helper_sems.items():
            for _ in range(counts[eng]):
                il.append(
                    mk_ev(
                        eng,
                        None,
                        mybir.SyncUpdate(
                            sync_type="semaphore",
                            id=h.num,
                            ant_name=h.name,
                            update_mode="sem-dec",
                            update_value=1,
                        ),
                    )
                )
                added = True
        if added:
            last_blk.instructions = il


NTILES = R // 128  # 25 gather tiles


def build_nc():
    nc = bass.Bass("TRN2", target_bir_lowering=False, debug=False)
    am_d = nc.declare_dram_parameter("am", [NM, E], BF16, isOutput=False)
    amT_d = nc.declare_dram_parameter("amT", [128, KE * SHARD], BF16, isOutput=False)
    mT_d = nc.declare_dram_parameter("mT", [128, KE * BS], BF16, isOutput=False)
    pwS_d = nc.declare_dram_parameter("pwS", [128, R], BF16, isOutput=False)
    # cols 0..NTILES-1: gather indices; NTILES: arange(128);
    # NTILES+1 / NTILES+2: this core's Tb-shard scatter row offsets
    idx_d = nc.declare_dram_parameter("idx", [128, NTILES + 3], I32, isOutput=False)
    rough_d = nc.declare_dram_parameter("rough", [1, R], F32, isOutput=False)
    w1a_d = nc.declare_dram_parameter("w1a", [E, HID], BF16, isOutput=False)
    w1b_d = nc.declare_dram_parameter("w1b", [E, HID], BF16, isOutput=False)
    w1c8_d = nc.declare_dram_parameter("w1c8", [128, KE * HID], F8E4, isOutput=False)
    w1d_d = nc.declare_dram_parameter("w1d", [PW, HID], BF16, isOutput=False)
    w2r_d = nc.declare_dram_parameter("w2r", [128, NT], BF16, isOutput=False)
    b1r_d = nc.declare_dram_parameter("b1r", [1, HID], BF16, isOutput=False)
    b2s_d = nc.declare_dram_parameter("b2s", [1, 1], F32, isOutput=False)
    out_d = nc.declare_dram_parameter("out", [1, R], F32, isOutput=True)

    helper_sems = {
        mybir.EngineType.SP: nc.alloc_semaphore("hoist_dma_sp"),
        mybir.EngineType.Pool: nc.alloc_semaphore("hoist_dma_pool"),
        mybir.EngineType.Activation: nc.alloc_semaphore("hoist_dma_act"),
        mybir.EngineType.DVE: nc.alloc_semaphore("hoist_dma_dve"),
    }

    with tile.TileContext(nc) as tc:
        with ExitStack() as ctx:
            const = ctx.enter_context(tc.tile_pool(name="const", bufs=1))
            dram = ctx.enter_context(tc.tile_pool(name="dram", bufs=1, space="DRAM"))

            w1c8_sb = const.tile([128, KE, HID], F8E4, tag="w1c8")
            wfused = const.tile([128, HID], BF16, tag="wfused")
            pwS_sb = const.tile([128, R], BF16, tag="pwS")
            mt_all = const.tile([128, KE, BS], BF16, tag="mt_all")
            it_all = const.tile([128, NTILES + 3], I32, tag="it_all")
            w2_sb = const.tile([128, NT], BF16, tag="w2")
            b2_sb = const.tile([1, 1], F32, tag="b2")
            identf = const.tile([128, 128], F32, tag="identf")
            identb = const.tile([128, 128], BF16, tag="identb")
            ident16 = const.tile([128, 128], F16, tag="ident16")
            ones_f = const.tile([1, BS], F32, tag="ones_f")
            ones1 = const.tile([1, BS], BF16, tag="ones1")

            make_identity(nc, identf[:])
            nc.vector.tensor_copy(identb[:], identf[:])
            nc.vector.tensor_copy(ident16[:], identf[:])
            nc.gpsimd.memset(ones_f[:], 1.0)
            nc.vector.tensor_copy(ones1[:], ones_f[:])

            # const loads on the Activation-engine DMA queue; idx first so
            # gathers can start immediately
            nc.scalar.dma_start(it_all[:], idx_d[:])
            nc.scalar.dma_start(mt_all[:].rearrange("p e n -> p (e n)"), mT_d[:])
            nc.scalar.dma_start(w2_sb[:], w2r_d[:])
            nc.scalar.dma_start(b2_sb[:], b2s_d[:])
            nc.scalar.dma_start(wfused[0:PW, :], w1d_d[:])

            def mts(e):
                return mt_all[:, e, :]

            tb_full = dram.tile([NM, HID], F16, tag="tbf", addr_space="Shared")
            tb_bounce = dram.tile([SHARD, HID], F16, tag="tbb")

            gb_pool = ctx.enter_context(tc.tile_pool(name="gb", bufs=NTILES))
            gt_pool = ctx.enter_context(tc.tile_pool(name="gt", bufs=16))
            abT_pool = ctx.enter_context(tc.tile_pool(name="abT", bufs=2))
            h_pool = ctx.enter_context(tc.tile_pool(name="h", bufs=4))
            hpre_pool = ctx.enter_context(tc.tile_pool(name="hpre", bufs=3 * NT))
            o_pool = ctx.enter_context(tc.tile_pool(name="o", bufs=2))
            rough_pool = ctx.enter_context(tc.tile_pool(name="rough", bufs=2))

            def emit_gb(c, rc, NC):
                gbs = []
                for t in range(NC // 128):
                    tg = rc // 128 + t
                    gb = gb_pool.tile([128, E], BF16, tag="gb", name=f"gb{c}_{t}")
                    nc.gpsimd.indirect_dma_start(
                        out=gb[:], out_offset=None, in_=am_d[:],
                        in_offset=bass.IndirectOffsetOnAxis(ap=it_all[:, tg : tg + 1], axis=0))
                    gbs.append(gb)
                abT = abT_pool.tile([128, KE, CHUNK], F8E4, tag="abT", name=f"abT{c}")
                return abT, gbs

            def emit_gt(c, rc, NC):
                gts = []
                for t in range(NC // 128):
                    tg = rc // 128 + t
                    gt = gt_pool.tile([128, E], F16, tag="gt", name=f"gt{c}_{t}")
                    nc.gpsimd.indirect_dma_start(
                        out=gt[:], out_offset=None, in_=tb_full[:],
                        in_offset=bass.IndirectOffsetOnAxis(ap=it_all[:, tg : tg + 1], axis=0))
                    gts.append(gt)
                return gts

            def transpose_unit(c, abT, gbs, t, e):
                tp = tp_pool.tile([128, 128], BF16, tag="tp", space="PSUM", name=f"tp{c}_{t}_{e}")
                nc.tensor.transpose(tp[:], gbs[t][:, 128 * e : 128 * (e + 1)], identb[:])
                sl = slice(128 * t, 128 * (t + 1))
                nc.vector.tensor_tensor(
                    out=abT[:, e, sl].rearrange("p (u m) -> p u m", m=BS),
                    in0=tp[:].rearrange("p (u m) -> p u m", m=BS),
                    in1=mt_all[:, e : e + 1, :].to_broadcast([128, 2, BS]),
                    op=mybir.AluOpType.mult)

            wa_pool = ctx.enter_context(tc.tile_pool(name="wa", bufs=8))
            b1_pool = ctx.enter_context(tc.tile_pool(name="b1p", bufs=1))
            b1_sb = b1_pool.tile([1, HID], BF16, tag="b1")
            nc.scalar.dma_start(b1_sb[:], b1r_d[:])
            was = []
            for k in range(KE):
                wa_t = wa_pool.tile([128, HID], BF16, tag="wa", name=f"wa{k}")
                nc.scalar.dma_start(wa_t[:], w1a_d[128 * k : 128 * (k + 1), :])
                was.append(wa_t)
            nc.scalar.dma_start(
                w1c8_sb[:].rearrange("p e n -> p (e n)"), w1c8_d[:]
            )
            nc.scalar.dma_start(pwS_sb[:], pwS_d[:])

            # ---- Phase T: Tb shard = amT_shard.T @ (32*W1b) -> fp16,
            # parity-packed so the whole 250-row shard scatters into the
            # SHARED table with ONE indirect DMA (row-pair granularity).
            HSH = SHARD // 2  # 125 row-pairs
            with tc.tile_pool(name="wb", bufs=8) as wb_pool, tc.tile_pool(
                name="ptb", bufs=4, space="PSUM"
            ) as ptb_pool, tc.tile_pool(name="tbsb", bufs=1) as tbsb_pool:
                amT_sb = tbsb_pool.tile([128, 2 * KE, HSH], BF16, tag="amT")
                nc.sync.dma_start(amT_sb[:].rearrange("p e m -> p (e m)"), amT_d[:])
                wbs = []
                for k in range(KE):
                    wb_t = wb_pool.tile([128, HID], BF16, tag="wb", name=f"wb{k}")
                    nc.sync.dma_start(wb_t[:], w1b_d[128 * k : 128 * (k + 1), :])
                    wbs.append(wb_t)
                tb_sb = tbsb_pool.tile([128, 2 * HID], F16, tag="tbsb")
                for j in range(2):
                    jsl = slice(512 * j, 512 * (j + 1))
                    for par in range(2):
                        ps_tb = ptb_pool.tile([128, 512], F32, tag="ptb", name=f"ps_tb{j}_{par}")[0:HSH, :]
                        for k in range(KE):
                            nc.tensor.matmul(
                                ps_tb[:], amT_sb[:, 2 * k + par, :], wbs[k][:, jsl],
                                start=(k == 0), stop=(k == KE - 1),
                            )
                        nc.vector.tensor_copy(
                            tb_sb[0:HSH, 1024 * par + 512 * j : 1024 * par + 512 * (j + 1)],
                            ps_tb[:],
                        )

                # one contiguous bounce write (row pairs == contiguous rows)
                nc.sync.dma_start(
                    tb_bounce[:].rearrange("(a b) c -> a (b c)", b=2),
                    tb_sb[0:HSH, :],
                )
                # all b-gathers BEFORE the collective (they flow while the
                # collective waits for its input), then the collective, then
                # all Tb gathers (their waits then block nothing upstream).
                gb_all = [emit_gb(c, _RCS[c], _CH[c]) for c in range(2)]
                nc.gpsimd.collective_compute(
                    "AllGather",
                    mybir.AluOpType.bypass,
                    replica_groups=[list(range(NCORES))],
                    ins=[tb_bounce[:]],
                    outs=[tb_full[:]],
                )
                for c in range(2, NCHUNK):
                    gb_all.append(emit_gb(c, _RCS[c], _CH[c]))
                gt_all = [emit_gt(c, _RCS[c], _CH[c]) for c in range(NCHUNK)]

            # ---- Phase A: Ta' = 32*(mentions @ W1a + b1) -> wfused[64:128, :]
            with tc.tile_pool(name="pta", bufs=2, space="PSUM") as pta_pool:
                for j in range(2):
                    jsl = slice(512 * j, 512 * (j + 1))
                    ps_ta = pta_pool.tile([128, 512], F32, tag="pta", name=f"ps_ta{j}")[0:BS, :]
                    nc.tensor.matmul(ps_ta[:], ones1[0:1, :], b1_sb[0:1, jsl], start=True, stop=False)
                    for k in range(KE):
                        nc.tensor.matmul(ps_ta[:], mts(k), was[k][:, jsl], start=False, stop=(k == KE - 1))
                    nc.vector.tensor_copy(wfused[PW : PW + BS, jsl], ps_ta[:])

            tp_pool = ctx.enter_context(tc.tile_pool(name="tp", bufs=3, space="PSUM"))
            psH = ctx.enter_context(tc.tile_pool(name="psH", bufs=3, space="PSUM"))
            psF = ctx.enter_context(tc.tile_pool(name="psF", bufs=2, space="PSUM"))

            # chunk 0: all transposes upfront
            for t in range(_CH[0] // 128):
                for e in range(KE):
                    transpose_unit(0, gb_all[0][0], gb_all[0][1], t, e)

            def emit_epilogue(c, rc, NC, ps_f):
                rough_t = rough_pool.tile([1, CHUNK], F32, tag="rough", name=f"ro{c}")
                nc.scalar.dma_start(rough_t[0:1, :NC], rough_d[0:1, rc : rc + NC])
                o_t = o_pool.tile([1, CHUNK], F32, tag="o", name=f"o{c}")
                nc.vector.tensor_tensor(out=o_t[0:1, :NC], in0=ps_f[0:1, :NC], in1=rough_t[0:1, :NC], op=mybir.AluOpType.add)
                nc.vector.tensor_scalar_add(o_t[0:1, :NC], o_t[0:1, :NC], b2_sb[0:1, 0:1])
                nc.sync.dma_start(out_d[0:1, rc : rc + NC], o_t[0:1, :NC])

            def emit_finish(c, rc, NC, n, ps_h, gts, ps_f):
                """Tb-injects + Lrelu + W2 for one (chunk, n-slice) group."""
                NCt = NC // 128
                nsl = slice(128 * n, 128 * (n + 1))
                for t in range(NCt):
                    nc.tensor.matmul(
                        ps_h[:, 128 * t : 128 * (t + 1)],
                        gts[t][:, nsl], ident16[:],
                        start=False, stop=(t == NCt - 1),
                    )
                h_t = h_pool.tile([128, CHUNK], BF16, tag="h", name=f"h{c}_{n}")
                nc.scalar.activation(
                    h_t[:, :NC], ps_h[:, :NC],
                    mybir.ActivationFunctionType.Lrelu,
                    alpha=ALPHA, scale=1.0 / SC,
                )
                nc.tensor.matmul(ps_f[0:1, :NC], w2_sb[:, n : n + 1], h_t[:, :NC], start=(n == 0), stop=(n == NT - 1))

            def emit_main(c, rc, NC, n, abT, defer):
                """pw/Ta-fused + fp8 DR matmuls for one (chunk, n) group.
                defer=True closes the group and parks it in bf16 h_pre."""
                nsl = slice(128 * n, 128 * (n + 1))
                halves = [(0, NC)]
                ps_h = psH.tile([128, CHUNK], F32, tag="ps_h", name=f"ps_h{c}_{n}")
                # open the bank full-width, then accumulate
                nc.tensor.matmul(ps_h[:, :NC], wfused[:, nsl], pwS_sb[:, rc : rc + NC], start=True, stop=False)
                for k2 in range(KE // 2):
                    esl = slice(2 * k2, 2 * k2 + 2)
                    last = k2 == KE // 2 - 1
                    for hi, (h0, hw) in enumerate(halves):
                        nc.tensor.matmul(
                            ps_h[:, h0 : h0 + hw],
                            w1c8_sb[:, esl, nsl],
                            abT[:, esl, h0 : h0 + hw],
                            start=False, stop=(defer and last and hi == len(halves) - 1),
                            perf_mode=mybir.MatmulPerfMode.DoubleRow,
                        )
                if defer:
                    h_pre = hpre_pool.tile([128, CHUNK], BF16, tag="hpre", name=f"hpre{c}_{n}")
                    nc.vector.tensor_copy(h_pre[:, :NC], ps_h[:, :NC])
                    return h_pre
                return ps_h

            # ---- chunks 0-1: DR+fused only, parked in h_pre (no dependency
            # on the collective); interleave next chunk's transposes
            N_DEFER = 3
            hpres = []
            for c in range(N_DEFER):
                rc, NC = _RCS[c], _CH[c]
                units = [(t, e) for t in range(_CH[c + 1] // 128) for e in range(KE)]
                per_group = (len(units) + NT - 1) // NT
                hp = []
                for n in range(NT):
                    hp.append(emit_main(c, rc, NC, n, gb_all[c][0], defer=True))
                    for _ in range(per_group):
                        if units:
                            t, e = units.pop(0)
                            transpose_unit(c + 1, gb_all[c + 1][0], gb_all[c + 1][1], t, e)
                hpres.append(hp)

            # ---- deferred finish of chunks 0-1: re-inject h_pre, add Tb,
            # activate, W2, epilogue
            for c in range(N_DEFER):
                rc, NC = _RCS[c], _CH[c]
                ps_f = psF.tile([1, CHUNK], F32, tag="ps_f", name=f"ps_f{c}")
                for n in range(NT):
                    ps_h = psH.tile([128, CHUNK], F32, tag="ps_h", name=f"ps_hd{c}_{n}")
                    nc.tensor.matmul(ps_h[:, :NC], identb[:], hpres[c][n][:, :NC], start=True, stop=False)
                    emit_finish(c, rc, NC, n, ps_h, gt_all[c], ps_f)
                emit_epilogue(c, rc, NC, ps_f)

            # ---- chunks 2+: normal fused flow
            for c in range(N_DEFER, NCHUNK):
                rc, NC = _RCS[c], _CH[c]
                gts = gt_all[c]
                if c + 1 < NCHUNK:
                    units = [(t, e) for t in range(_CH[c + 1] // 128) for e in range(KE)]
                else:
                    units = []
                per_group = (len(units) + NT - 1) // NT if units else 0
                ps_f = psF.tile([1, CHUNK], F32, tag="ps_f", name=f"ps_f{c}")
                for n in range(NT):
                    ps_h = emit_main(c, rc, NC, n, gb_all[c][0], defer=False)
                    emit_finish(c, rc, NC, n, ps_h, gts, ps_f)
                    for _ in range(per_group):
                        if units:
                            t, e = units.pop(0)
                            transpose_unit(c + 1, gb_all[c + 1][0], gb_all[c + 1][1], t, e)
                emit_epilogue(c, rc, NC, ps_f)

    _redistribute_waits(nc, helper_sems)
    return nc


_NC_CACHE = None


def _get_nc():
    global _NC_CACHE
    if _NC_CACHE is None:
        _NC_CACHE = build_nc()
    return _NC_CACHE


BF = ml_dtypes.bfloat16
F8 = ml_dtypes.float8_e4m3


def make_in_maps(
    all_mentions,
    mentions_batch,
    pw_batch,
    top_indices_batch,
    top_rough_scores_batch,
    W1,
    b1,
    W2,
    b2,
):
    am = np.asarray(all_mentions, np.float32)
    men = np.asarray(mentions_batch, np.float32)
    pw = np.asarray(pw_batch, np.float32)
    idx = np.asarray(top_indices_batch).astype(np.int32)
    rough = np.asarray(top_rough_scores_batch, np.float32)
    W1 = np.asarray(W1, np.float32)
    b1 = np.asarray(b1, np.float32)
    W2 = np.asarray(W2, np.float32)
    b2 = np.asarray(b2, np.float32)

    am_bf = am.astype(BF)
    w1a = np.ascontiguousarray((SC * W1[0:E]).astype(BF))
    w1b = np.ascontiguousarray((SC * W1[E : 2 * E]).astype(BF))
    w1c8 = np.ascontiguousarray(
        (SC * W1[2 * E : 3 * E]).reshape(KE, 128, HID).transpose(1, 0, 2).reshape(128, KE * HID)
    ).astype(F8)
    w1d = np.ascontiguousarray((SC * W1[3 * E : 3 * E + PW]).astype(BF))
    w2r = np.ascontiguousarray(W2[:, 0].reshape(NT, 128).T.astype(BF))
    b1r = np.ascontiguousarray((SC * b1).reshape(1, HID).astype(BF))
    b2s = np.ascontiguousarray(b2.reshape(1, 1))
    S = np.tile(np.eye(BS, dtype=np.float32), (1, A))

    in_maps = []
    for c in range(NCORES):
        sl = slice(c * BS, (c + 1) * BS)
        # [128, KE*BS]: mT[p, k*BS+m] = men[c*BS+m, 128k+p]
        mT = np.ascontiguousarray(
            men[sl].T.astype(BF).reshape(KE, 128, BS).transpose(1, 0, 2).reshape(128, KE * BS)
        )
        # parity-packed: amT[p, (2k+par)*125 + m'] = am[c*SHARD + 2m'+par, 128k+p]
        sh = am_bf[c * SHARD : (c + 1) * SHARD].T.reshape(KE, 128, SHARD // 2, 2)
        amT = np.ascontiguousarray(
            sh.transpose(1, 0, 3, 2).reshape(128, KE * SHARD)
        )
        pwT = pw[sl].transpose(2, 1, 0).reshape(PW, R)
        pwS = np.ascontiguousarray(np.concatenate([pwT, S], axis=0).astype(BF))
        # [128, NTILES+3] column-major per 128-row tile, then arange and
        # the Tb-shard scatter row offsets for this core
        cols = np.empty((128, NTILES + 3), np.int32)
        cols[:, :NTILES] = idx[sl].T.reshape(R).reshape(NTILES, 128).T
        ar = np.arange(128, dtype=np.int32)
        cols[:, NTILES] = ar
        # row-PAIR offsets into the [1000, 2048] view of the Tb table
        cols[:, NTILES + 1] = np.minimum(c * (SHARD // 2) + ar, NM // 2 - 1)
        cols[:, NTILES + 2] = 0
        idx_r = np.ascontiguousarray(cols)
        rough_r = np.ascontiguousarray(rough[sl].T.reshape(1, R))
        in_maps.append(
            dict(
                am=am_bf,
                amT=amT,
                mT=mT,
                pwS=pwS,
                idx=idx_r,
                rough=rough_r,
                w1a=w1a,
                w1b=w1b,
                w1c8=w1c8,
                w1d=w1d,
                w2r=w2r,
                b1r=b1r,
                b2s=b2s,
            )
        )
    return in_maps


def assemble_output(results):
    scores = np.empty((BATCH, A), np.float32)
    for c in range(NCORES):
        score_r = np.asarray(results[c]["out"]).reshape(A, BS)
        scores[c * BS : (c + 1) * BS, :] = score_r.T
    out = np.empty((BATCH, A + 1), np.float32)
    out[:, 0] = EPSILON
    out[:, 1:] = scores
    return out


def kernel(**inputs):
    nc = _get_nc()
    in_maps = make_in_maps(**inputs)
    res = run_bass_kernel_spmd(nc, in_maps, core_ids=list(range(NCORES)))
    return assemble_output(res.results)


if __name__ == "__main__":
    nc = build_nc()
    print("built ok")
